# revision 63
# baseline (speedup 1.0000x reference)
"""Trainium2 Bass kernel for nn_AttentionModule (sparse_attention).

Reference computation:
  q = tanh(einsum('hde,be->hbd', Query, x))          H=8 D=256 E=1536
  k = tanh(einsum('hdf,blf->hbld', Key, bank))       B=64 L=256 F=768
  s = einsum('hbld,hbd->hbl', k, q)  masked softmax over l
  out = LeakyReLU_0.4(einsum('hbl,blf->bhf', attn, bank))

Strategy (hybrid shard: 4 batch-groups x 2 head-groups over 8 cores):
 * Each core owns 16 b's (8 sorted pairs) and 4 heads.  This halves the
   replicated Query/Key DMA vs pure batch-parallel (the serial DMA wire,
   ~0.36 MB/us, binds): per-core input drops 12.9 -> ~11.3 MB, and the
   3.15 MB Query stream lands by ~17 us, so the score/softmax pipeline
   never waits on it.
 * Mask compaction: the 0/1 mask keeps <=~152 of 256 bank columns per b;
   the host gathers unmasked columns, sorts b's by count, pads each
   pair-slot to the max of its 8 ranked b's.  Padding columns get a -1e4
   additive score bias (exp -> 0) via an extra matmul.
 * The dominant k-matmul runs as error-compensated fp8 (e4m3): with
   Key*64 ~ K8 + Kr and bank*16 ~ B8 + Br, kraw = K8B8 + K8Br + KrB8
   (the fp8*fp8 residual cross term is negligible).  All three terms
   share one power-of-two scale, folded into the tanh eviction's
   `scale`.  Each product pair runs as a DoubleRow matmul.
 * Narrow dims ride in the moving dimension: q, score, and emb matmuls
   cost ap_size 16/1/4 per instruction instead of 256-512.
 * score/softmax/emb runs as FOUR quad-chains (4 b's = 2 bp-pairs per
   PSUM tile, rows padded to the quad max): same chain count as the
   well-pipelined batch-parallel version, half the Query bytes.  The
   last two k phases run joint (both bps per head) so quad3's inputs
   finish early; quad2's softmax rides inside that phase.
 * Softmax skips max-subtraction (|score| < 40, exp in bf16 is safe);
   1/z is broadcast to [f, h] via a ones-matrix matmul and applied with
   LeakyReLU via one DVE multiply.  Outputs leave in two f16 out-DMAs
   (quads 0-2 mid-kernel; only quad3's small transfer sits on the tail).
"""

import os
import numpy as np
import ml_dtypes

import concourse.bass as bass  # noqa: F401
import concourse.mybir as mybir
import concourse.tile as tile
from concourse import bacc, bass_utils

F32 = mybir.dt.float32
F16 = mybir.dt.float16
BF16 = mybir.dt.bfloat16
FP8 = mybir.dt.float8e4
AF = mybir.ActivationFunctionType
ALU = mybir.AluOpType
DR = mybir.MatmulPerfMode.DoubleRow

H, D, E, F = 8, 256, 1536, 768
B, L = 64, 256
NCORES = 8
GB, GH = 4, 2              # batch groups x head groups
HL = H // GH               # 4 local heads
BPC = B // GB              # 16 b's per core
NBP = BPC // 2             # 8 b-pairs per core
NQ = NBP // 2              # 4 score/softmax quads (4 b's each)
EC, FC, DC = E // 128, F // 128, D // 128   # 12, 6, 2
# Per-bp padded unmasked-column counts (host sorts 64 b's by count; bp_j
# takes ranks [8j, 8j+8)).  Defaults match the fixed harness input.
LPS_DEFAULT = (152, 136, 132, 130, 128, 126, 124, 120)
SK, SB = 64.0, 16.0        # fp8 pre-scales for Key / bank (powers of two)


def _build_program(lps=LPS_DEFAULT):
    assert all(lp % 2 == 0 for lp in lps)
    lhs_ = [lp // 2 for lp in lps]     # l-chunks: two per b
    lpps = [2 * lp for lp in lps]      # (b2, l') columns per (h, dc) group
    lqs = [lhs_[2 * j] for j in range(NQ)]    # quad row counts (max of pair)
    kt_cols = 2 * FC * D               # per-h Key cols ([K8, Kr] streams)
    bkt_cols = [2 * FC * w for w in lpps]     # per-bp bankT cols
    bkt_off = np.cumsum([0] + bkt_cols).tolist()
    # bkn/sbias rows are padded to the quad max so one softmax chain can
    # cover 4 b's; pad rows carry -1e4 bias (exp -> 0) / zero bank rows
    bkn_off = np.cumsum([0] + [2 * lqs[bp // 2] for bp in range(NBP)]).tolist()
    sb_off = np.cumsum([0] + [4 * lqs[bp // 2] for bp in range(NBP)]).tolist()
    tanh_scale = 1.0 / (SK * SB)

    nc = bacc.Bacc("TRN2", target_bir_lowering=False, debug=False,
                   enable_asserts=False, num_devices=NCORES)
    qt = nc.dram_tensor("qt", [HL, 128, EC * D], F16, kind="ExternalInput").ap()
    xt = nc.dram_tensor("xt", [128, EC * BPC], F16, kind="ExternalInput").ap()
    kt = nc.dram_tensor("kt", [HL, 128, kt_cols], FP8, kind="ExternalInput").ap()
    bkt = nc.dram_tensor("bkt", [128, bkt_off[-1]], FP8, kind="ExternalInput").ap()
    bkn = nc.dram_tensor("bkn", [bkn_off[-1], 2 * F], BF16, kind="ExternalInput").ap()
    sbias = nc.dram_tensor("sbias", [1, sb_off[-1]], F32, kind="ExternalInput").ap()
    # out cols: (quad, i4=(bp2, b2), fc, h)
    out = nc.dram_tensor("out", [128, NQ * 4 * FC * HL], F16,
                         kind="ExternalOutput").ap()

    with tile.TileContext(nc) as tc:
        with tc.tile_pool(name="const", bufs=1) as cpool, \
             tc.tile_pool(name="weights", bufs=1) as wpool, \
             tc.tile_pool(name="bktp", bufs=1) as bpool, \
             tc.tile_pool(name="bknp", bufs=1) as npool, \
             tc.tile_pool(name="ksb", bufs=1) as kpool, \
             tc.tile_pool(name="small", bufs=4) as spool, \
             tc.tile_pool(name="psK", bufs=4, space="PSUM") as psK, \
             tc.tile_pool(name="psQ", bufs=1, space="PSUM") as psQ, \
             tc.tile_pool(name="psS", bufs=3, space="PSUM") as psS:

            # ---------------- SBUF tiles ----------------------------------
            xt_sb = cpool.tile([128, EC * BPC], F16)
            kt_sb = [wpool.tile([128, kt_cols], FP8, name=f"kt{h}", tag=f"kt{h}")
                     for h in range(HL)]
            qt_sb = [wpool.tile([128, EC * D], F16, name=f"qt{h}", tag=f"qt{h}")
                     for h in range(HL)]
            bkt_t = [bpool.tile([128, bkt_cols[bp]], FP8,
                                name=f"bkt{bp}", tag=f"bkt{bp}")
                     for bp in range(NBP)]
            bkn_t = [[npool.tile([lqs[bp // 2], 2 * F], BF16,
                                 name=f"bkn{bp}_{b2}", tag=f"bkn{bp}_{b2}")
                      for b2 in range(2)] for bp in range(NBP)]
            sb_sb = cpool.tile([1, sb_off[-1]], F32)
            # f16 output: final values are O(1), so f16 (0.05% rel) halves
            # the tail-critical out-DMA transfers; host upcasts
            o2a = cpool.tile([128, (NQ - 1) * 4 * FC * HL], F16)
            o2b = cpool.tile([128, 4 * FC * HL], F16)
            onesb = cpool.tile([1, BPC], F32)
            ones_mat = cpool.tile([lqs[0], 128], BF16)
            q_sb = cpool.tile([128, 128], F16)

            # ---------------- DMA: priority order -------------------------
            def dma_bkt(bp, s=None):
                o = bkt_off[bp]
                w = bkt_cols[bp]
                if s is None:
                    nc.sync.dma_start(bkt_t[bp][:], bkt[:, o:o + w])
                else:
                    h2 = w // 2
                    nc.sync.dma_start(bkt_t[bp][:, s * h2:(s + 1) * h2],
                                      bkt[:, o + s * h2:o + (s + 1) * h2])

            # kt0 K8-half and bkt0/1 B8-halves first so the warm-phase T1
            # matmuls (k0+k1 interleaved per head) start early
            hk = kt_cols // 2
            nc.sync.dma_start(kt_sb[0][:, 0:hk], kt[0, :, 0:hk])
            dma_bkt(0, 1)
            dma_bkt(1, 1)
            nc.sync.dma_start(kt_sb[0][:, hk:2 * hk], kt[0, :, hk:2 * hk])
            dma_bkt(0, 0)
            dma_bkt(1, 0)
            nc.sync.dma_start(kt_sb[1][:, 0:hk], kt[1, :, 0:hk])
            nc.sync.dma_start(kt_sb[1][:, hk:2 * hk], kt[1, :, hk:2 * hk])
            nc.sync.dma_start(xt_sb[:], xt)
            nc.vector.memset(onesb[:], 1.0)
            nc.vector.memset(ones_mat[:], 1.0)
            nc.sync.dma_start(kt_sb[2][:], kt[2])
            dma_bkt(2)
            nc.sync.dma_start(kt_sb[3][:], kt[3])
            dma_bkt(3)
            nc.sync.dma_start(qt_sb[0][:], qt[0])
            dma_bkt(4)
            nc.sync.dma_start(qt_sb[1][:], qt[1])
            dma_bkt(5)
            nc.sync.dma_start(qt_sb[2][:], qt[2])
            nc.sync.dma_start(qt_sb[3][:], qt[3])
            nc.sync.dma_start(sb_sb[:], sbias)
            dma_bkt(6)
            dma_bkt(7)
            for bp in range(NBP):
                for b2 in range(2):
                    r = bkn_off[bp] + b2 * lqs[bp // 2]
                    nc.sync.dma_start(bkn_t[bp][b2][:],
                                      bkn[r:r + lqs[bp // 2]])

            # ---------------- k = tanh(Key @ bankT), all bps --------------
            k_sb = {}

            def k_phase(bps, warm=False, post_h=None, tail_quad=None):
                def t1_mms(bp, h, ps):
                    lpp = lpps[bp]
                    vb = bkt_t[bp][:].rearrange("p (s ft c) -> p s ft c",
                                                s=2, ft=FC)
                    vk = kt_sb[h][:].rearrange("p (s ft d) -> p s ft d",
                                               s=2, ft=FC)
                    for dc in range(DC):
                        g = ps[dc][:, 0:lpp]
                        for p in range(FC // 2):
                            nc.tensor.matmul(
                                g,
                                vk[:, 0, 2 * p:2 * p + 2,
                                   dc * 128:(dc + 1) * 128],
                                vb[:, 1, 2 * p:2 * p + 2],
                                start=(p == 0), stop=False, perf_mode=DR)

                def cross_evict(bp, h, ps):
                    lpp = lpps[bp]
                    vb = bkt_t[bp][:].rearrange("p (s ft c) -> p s ft c",
                                                s=2, ft=FC)
                    vk = kt_sb[h][:].rearrange("p (s ft d) -> p s ft d",
                                               s=2, ft=FC)
                    for dc in range(DC):
                        g = ps[dc][:, 0:lpp]
                        # cross terms: K8.Br + Kr.B8 per f-tile
                        for ft in range(FC):
                            nc.tensor.matmul(
                                g, vk[:, :, ft, dc * 128:(dc + 1) * 128],
                                vb[:, :, ft],
                                start=False, stop=(ft == FC - 1),
                                perf_mode=DR)
                        # per-dc eviction into a per-dc tile: dc0 drains
                        # while dc1 fills, and (deps being tile-granular)
                        # score matmuls of dc0 can fire before the dc1
                        # eviction lands
                        kt_out = kpool.tile([128, lpp], F16,
                                            name=f"k{bp}_{h}_{dc}",
                                            tag=f"k{bp}_{h}_{dc}")
                        nc.scalar.activation(
                            kt_out[:, 0:lpp],
                            ps[dc][:, 0:lpp],
                            AF.Tanh, scale=tanh_scale)
                        k_sb[(bp, h, dc)] = kt_out

                start_h = 0
                if warm:
                    # four T1 groups lead (they need only the K8/B8
                    # slices; the residual streams for the crosses land
                    # while they run).  The third group borrows a psS
                    # buffer pair (idle until the score phase).
                    b0, b1 = bps[0], bps[1]
                    tA = [psK.tile([128, 512], F32, name="psk", tag="psk")
                          for _ in range(DC)]
                    t1_mms(b0, 0, tA)
                    tB = [psK.tile([128, 512], F32, name="psk", tag="psk")
                          for _ in range(DC)]
                    t1_mms(b1, 0, tB)
                    tC = [psS.tile([128, 512], F32, name="mix", tag="mix")
                          for _ in range(DC)]
                    t1_mms(b0, 1, tC)
                    cross_evict(b0, 0, tA)
                    tD = [psK.tile([128, 512], F32, name="psk", tag="psk")
                          for _ in range(DC)]
                    t1_mms(b1, 1, tD)
                    cross_evict(b1, 0, tB)
                    cross_evict(b0, 1, tC)
                    cross_evict(b1, 1, tD)
                    start_h = 2
                for h in range(start_h, HL):
                    for bp in bps:
                        ps = [psK.tile([128, 512], F32,
                                       name="psk", tag="psk")
                              for _ in range(DC)]
                        t1_mms(bp, h, ps)
                        cross_evict(bp, h, ps)
                    if tail_quad is not None and h >= 1:
                        # the last quad's score matmuls ride one head
                        # behind the joint phase's own evictions
                        qd, qps = tail_quad
                        for bp2 in range(2):
                            score_qh(qd, qps, bp2, h - 1)
                    if post_h and h in post_h:
                        post_h[h]()

            # k0+k1 interleaved per head (2.3us of PE work per kt[h]
            # arrival so the lead-in is never DMA-starved)
            k_phase([0, 1], warm=True)
            k_phase([2])
            k_phase([3])

            # ---------------- q = tanh(Query @ x), transposed -------------
            # qt (3.15 MB) has streamed in behind the k inputs by now
            psq = psQ.tile([128, 512], F32)
            for h in range(HL):
                vq = qt_sb[h][:].rearrange("p (ec d) -> p ec d", ec=EC)
                for dc in range(DC):
                    g = psq[:, (h * DC + dc) * BPC:(h * DC + dc + 1) * BPC]
                    for ec in range(EC):
                        nc.tensor.matmul(
                            g, vq[:, ec, dc * 128:(dc + 1) * 128],
                            xt_sb[:, ec * BPC:(ec + 1) * BPC],
                            start=(ec == 0), stop=(ec == EC - 1))
            nc.scalar.activation(q_sb[:], psq[:, 0:128], AF.Tanh)

            # ---------------- score / softmax / emb per QUAD --------------
            # One chain covers 4 b's (bps 2j, 2j+1), rows padded to the
            # quad max (pad rows get -1e4 bias -> exp 0, zero bank rows).
            # score cols: (i4=(bp2, b2), lc, h) -> 8 groups of HL
            def score_qh(qd, ps, bp2, h):
                """One head's score matmuls for one bp of a quad (+bias
                matmuls at h==0)."""
                lq = lqs[qd]
                bp = 2 * qd + bp2
                lh, lp, lpp = lhs_[bp], lps[bp], lpps[bp]
                for b2 in range(2):
                    i4 = bp2 * 2 + b2
                    for lc in range(2):
                        col = (i4 * 2 + lc) * HL
                        if h == 0:
                            boff = sb_off[bp] + (b2 * 2 + lc) * lq
                            nc.tensor.matmul(ps[0:lq, col:col + HL],
                                             sb_sb[:, boff:boff + lq],
                                             onesb[:, 0:HL],
                                             start=True, stop=False)
                        for dc in range(DC):
                            nc.tensor.matmul(
                                ps[0:lh, col + h:col + h + 1],
                                k_sb[(bp, h, dc)][:, b2 * lp + lc * lh:
                                                  b2 * lp + lc * lh + lh],
                                q_sb[:, (h * DC + dc) * BPC + bp * 2 + b2:
                                     (h * DC + dc) * BPC + bp * 2 + b2 + 1],
                                start=False,
                                stop=(h == HL - 1 and dc == DC - 1))

            def score_exp(qd, ps):
                lq = lqs[qd]
                exp_t = spool.tile([lqs[0], 8 * HL], BF16,
                                   name="exp", tag="exp")
                nc.scalar.activation(exp_t[0:lq, :], ps[0:lq, 0:8 * HL],
                                     AF.Exp)
                return ps, exp_t

            def score_part(qd, ps_tile=None):
                ps = (ps_tile if ps_tile is not None
                      else psS.tile([128, 512], F32, name="mix", tag="mix"))
                for bp2 in range(2):
                    for h in range(HL):
                        score_qh(qd, ps, bp2, h)
                return score_exp(qd, ps)

            def rest_part(qd, ps, exp_t):
                lq = lqs[qd]
                # z[i4, h] (cols 64:80): the ones-MATRIX lhsT emits column
                # sums broadcast across all 128 partitions
                ev = exp_t[0:lq, :].rearrange("p (i4 lc h) -> p i4 lc h",
                                              i4=4, lc=2)
                for lc in range(2):
                    nc.tensor.matmul(ps[:, 64:64 + 4 * HL],
                                     ones_mat[0:lq, :], ev[:, :, lc],
                                     start=(lc == 0), stop=(lc == 1))
                rz = spool.tile([128, 4 * HL], F32, name="rz", tag="rz")
                nc.vector.reciprocal(rz[:], ps[:, 64:64 + 4 * HL])
                # emb[f, (i4, fc, h)] (cols 192:288)
                for bp2 in range(2):
                    bp = 2 * qd + bp2
                    for b2 in range(2):
                        i4 = bp2 * 2 + b2
                        for fc in range(FC):
                            col = 192 + (i4 * FC + fc) * HL
                            for lc in range(2):
                                nc.tensor.matmul(
                                    ps[:, col:col + HL],
                                    bkn_t[bp][b2][:, lc * F + fc * 128:
                                                  lc * F + fc * 128 + 128],
                                    exp_t[0:lq, (i4 * 2 + lc) * HL:
                                          (i4 * 2 + lc + 1) * HL],
                                    start=(lc == 0), stop=(lc == 1))
                o1 = spool.tile([128, 4 * FC * HL], F32, name="o1", tag="o1")
                w = 4 * FC * HL
                o2 = (o2b[:, 0:w] if qd == NQ - 1
                      else o2a[:, qd * w:(qd + 1) * w])
                # LeakyReLU commutes with the positive 1/z: Prelu the raw
                # emb on Act in parallel with the z/recip chain; one DVE
                # multiply finishes
                nc.scalar.activation(o1[:], ps[:, 192:192 + w], AF.Prelu,
                                     alpha=0.4)
                vb = rz[:].rearrange(
                    "p (i4 one h) -> p i4 one h", i4=4,
                    one=1).broadcast_to([128, 4, FC, HL])
                nc.vector.tensor_mul(
                    o2.rearrange("p (i4 fc h) -> p i4 fc h", i4=4, fc=FC),
                    o1[:].rearrange("p (i4 fc h) -> p i4 fc h", i4=4, fc=FC),
                    vb)
                if qd == NQ - 2:
                    # quads 0-2 stream out while quad3 still computes
                    nc.sync.dma_start(out[:, 0:(NQ - 1) * w], o2a[:])
                elif qd == NQ - 1:
                    # only quad3's small transfer sits on the tail
                    nc.sync.dma_start(out[:, (NQ - 1) * w:NQ * w], o2b[:])

            # quads 0/1 score+softmax overlap k4/k5; quad2's inputs finish
            # at k5 so its chain rides inside the joint k6+k7 phase; quad3
            # scores after it (pending/rest pipeline shape)
            k_phase([4])
            # quad0/1 scores issue right after their last input phase so
            # their softmax chains drain a phase earlier; quad2's scores
            # ride inside k5 (bp4's k is done, bp5 one head behind)
            cur0 = (0, *score_part(0))
            ps2t = psS.tile([128, 512], F32, name="mix", tag="mix")
            k_phase([5], tail_quad=(2, ps2t))
            cur1 = (1, *score_part(1))
            rest_part(*cur0)
            for bp2 in range(2):
                score_qh(2, ps2t, bp2, HL - 1)
            cur2 = (2, *score_exp(2, ps2t))
            rest_part(*cur1)
            k_phase([6, 7], post_h={1: (lambda: rest_part(*cur2))},
                    tail_quad=(NQ - 1, psq))
            # tail: last head's scores, exp, softmax/emb/out for quad3
            for bp2 in range(2):
                score_qh(NQ - 1, psq, bp2, HL - 1)
            rest_part(NQ - 1, *score_exp(NQ - 1, psq))

    nc.finalize()
    return nc


def _slot_plan(mask):
    """Sort b's by unmasked count (desc); bp_j takes ranks [8j, 8j+8).
    Returns (perm, lps): perm[slot] = original b, slot = gb*BPC + j*2 + b2."""
    counts = mask.sum(axis=1)
    order = np.argsort(-counts, kind="stable")
    perm = np.empty(B, dtype=np.int64)
    for j in range(NBP):
        grp = order[8 * j:8 * (j + 1)]
        for gb in range(GB):
            perm[gb * BPC + j * 2] = grp[2 * gb]
            perm[gb * BPC + j * 2 + 1] = grp[2 * gb + 1]
    lps = tuple(max(int(2 * ((counts[order[8 * j]] + 1) // 2)), 8)
                for j in range(NBP))
    return perm, lps


def _host_prep(x, bank, mask, Query, Key, perm, lps):
    x = np.asarray(x, dtype=np.float32)
    bank = np.asarray(bank, dtype=np.float32)
    mask = np.asarray(mask)
    Query = np.asarray(Query, dtype=np.float32)
    Key = np.asarray(Key, dtype=np.float32)
    e4 = ml_dtypes.float8_e4m3
    lhs_ = [lp // 2 for lp in lps]
    lqs = [lhs_[2 * j] for j in range(len(lps) // 2)]

    # q path: f16, host-transposed; per head-group slice
    xs = x[perm]
    qt_full = np.ascontiguousarray(Query.transpose(0, 2, 1)).reshape(
        H, EC, 128, D).transpose(0, 2, 1, 3).reshape(H, 128, EC * D)
    qt_full = qt_full.astype(np.float16)

    Ks = Key * SK
    K8 = Ks.astype(e4)
    Kr = (Ks - K8.astype(np.float32)).astype(e4)

    def swz_key(Kt):  # [H, D, F] -> [H, 128(f), FC, D]
        t = np.ascontiguousarray(Kt.transpose(0, 2, 1))
        return t.reshape(H, FC, 128, D).transpose(0, 2, 1, 3)

    kt_full = np.stack([swz_key(K8.astype(np.float32)),
                        swz_key(Kr.astype(np.float32))], axis=2)
    kt_full = kt_full.reshape(H, 128, 2 * FC * D).astype(e4)

    # per-(batch-group, bp) compacted bank streams
    bkt_cols = sum(2 * FC * 2 * lp for lp in lps)
    gb_data = []
    for gb in range(GB):
        bkt_c = np.zeros((128, bkt_cols), dtype=e4)
        bkn_rows = []
        sb_c = []
        col = 0
        for j in range(NBP):
            lp, lh, lq = lps[j], lhs_[j], lqs[j // 2]
            bc = np.zeros((2, lp, F), dtype=np.float32)
            # bias rows padded to the quad max (pad rows -> -1e4)
            bias = np.full((2, 2, lq), -10000.0, dtype=np.float32)
            for b2 in range(2):
                bsrc = perm[gb * BPC + j * 2 + b2]
                idx = np.nonzero(mask[bsrc])[0]
                bc[b2, :len(idx)] = bank[bsrc, idx]
                # column c of this b maps to row c%lh, chunk lc=c//lh;
                # valid rows per lc chunk:
                n = len(idx)
                for lc in range(2):
                    good = max(0, min(lh, n - lc * lh))
                    bias[b2, lc, :good] = 0.0
            # bankT swizzle: [2, lp, F] -> [128(f), s, FC, 2, lp]
            t = np.ascontiguousarray(bc.transpose(0, 2, 1))     # [2, F, lp]
            t = t.reshape(2, FC, 128, lp).transpose(2, 1, 0, 3)  # [128,FC,2,lp]
            ts = t * SB
            t8 = ts.astype(e4)
            tr = (ts - t8.astype(np.float32)).astype(e4)
            blk = np.stack([tr, t8.astype(e4)], axis=1).reshape(
                128, 2 * FC * 2 * lp)
            w = 2 * FC * 2 * lp
            bkt_c[:, col:col + w] = blk
            col += w
            # bkn rows [2*lq, 2F] per bp (b2-major): rows beyond lh zero
            br = np.zeros((2, lq, 2 * F), dtype=np.float32)
            bcq = bc.reshape(2, 2, lh, F)   # [b2, lc, lh, F]
            br[:, :lh, 0:F] = bcq[:, 0]
            br[:, :lh, F:2 * F] = bcq[:, 1]
            bkn_rows.append(br.reshape(2 * lq, 2 * F))
            sb_c.append(bias.reshape(4 * lq))
        xt_gb = np.ascontiguousarray(
            xs[gb * BPC:(gb + 1) * BPC].T.reshape(EC, 128, BPC)
            .transpose(1, 0, 2).reshape(128, EC * BPC)).astype(np.float16)
        gb_data.append({
            "xt": xt_gb,
            "bkt": bkt_c,
            "bkn": np.ascontiguousarray(np.concatenate(bkn_rows, axis=0))
            .astype(ml_dtypes.bfloat16),
            "sbias": np.concatenate(sb_c)[None, :].astype(np.float32),
        })

    in_maps = []
    for c in range(NCORES):
        gb, gh = c // GH, c % GH
        m = dict(gb_data[gb])
        m["qt"] = qt_full[gh * HL:(gh + 1) * HL]
        m["kt"] = kt_full[gh * HL:(gh + 1) * HL]
        in_maps.append(m)
    return in_maps


_NC_CACHE = {}


def kernel(x, bank, mask, Query, Key):
    mask = np.asarray(mask)
    perm, lps = _slot_plan(mask)
    if lps not in _NC_CACHE:
        _NC_CACHE[lps] = _build_program(lps)
    nc = _NC_CACHE[lps]
    in_maps = _host_prep(x, bank, mask, Query, Key, perm, lps)

    trace = os.environ.get("KERNEL_TRACE", "0") == "1"
    res = bass_utils.run_bass_kernel_spmd(nc, in_maps,
                                          core_ids=list(range(NCORES)),
                                          trace=trace)
    full = np.empty((B, H, F), dtype=np.float32)
    for c, r in enumerate(res.results):
        gb, gh = c // GH, c % GH
        a = r["out"].astype(np.float32).reshape(128, NQ, 2, 2, FC, HL)
        # [p, quad, bp2, b2, fc, h] -> slot (j=quad*2+bp2, b2) -> [BPC,HL,F]
        a = a.transpose(1, 2, 3, 5, 4, 0).reshape(BPC, HL, F)
        full[perm[gb * BPC:(gb + 1) * BPC], gh * HL:(gh + 1) * HL] = a
    return np.ascontiguousarray(full)


# revision 67
# speedup vs baseline: 1.0003x; 1.0003x over previous
"""Trainium2 Bass kernel for nn_AttentionModule (sparse_attention).

Reference computation:
  q = tanh(einsum('hde,be->hbd', Query, x))          H=8 D=256 E=1536
  k = tanh(einsum('hdf,blf->hbld', Key, bank))       B=64 L=256 F=768
  s = einsum('hbld,hbd->hbl', k, q)  masked softmax over l
  out = LeakyReLU_0.4(einsum('hbl,blf->bhf', attn, bank))

Strategy (hybrid shard: 4 batch-groups x 2 head-groups over 8 cores):
 * Each core owns 16 b's (8 sorted pairs) and 4 heads.  This halves the
   replicated Query/Key DMA vs pure batch-parallel (the serial DMA wire,
   ~0.36 MB/us, binds): per-core input drops 12.9 -> ~11.3 MB, and the
   3.15 MB Query stream lands by ~17 us, so the score/softmax pipeline
   never waits on it.
 * Mask compaction: the 0/1 mask keeps <=~152 of 256 bank columns per b;
   the host gathers unmasked columns, sorts b's by count, pads each
   pair-slot to the max of its 8 ranked b's.  Padding columns get a -1e4
   additive score bias (exp -> 0) via an extra matmul.
 * The dominant k-matmul runs as error-compensated fp8 (e4m3): with
   Key*64 ~ K8 + Kr and bank*16 ~ B8 + Br, kraw = K8B8 + K8Br + KrB8
   (the fp8*fp8 residual cross term is negligible).  All three terms
   share one power-of-two scale, folded into the tanh eviction's
   `scale`.  Each product pair runs as a DoubleRow matmul.
 * Narrow dims ride in the moving dimension: q, score, and emb matmuls
   cost ap_size 16/1/4 per instruction instead of 256-512.
 * score/softmax/emb runs as FOUR quad-chains (4 b's = 2 bp-pairs per
   PSUM tile, rows padded to the quad max): same chain count as the
   well-pipelined batch-parallel version, half the Query bytes.  The
   last two k phases run joint (both bps per head) so quad3's inputs
   finish early; quad2's softmax rides inside that phase.
 * Softmax skips max-subtraction (|score| < 40, exp in bf16 is safe);
   1/z is broadcast to [f, h] via a ones-matrix matmul and applied with
   LeakyReLU via one DVE multiply.  Outputs leave in two f16 out-DMAs
   (quads 0-2 mid-kernel; only quad3's small transfer sits on the tail).
"""

import os
import numpy as np
import ml_dtypes

import concourse.bass as bass  # noqa: F401
import concourse.mybir as mybir
import concourse.tile as tile
from concourse import bacc, bass_utils

F32 = mybir.dt.float32
F16 = mybir.dt.float16
BF16 = mybir.dt.bfloat16
FP8 = mybir.dt.float8e4
AF = mybir.ActivationFunctionType
ALU = mybir.AluOpType
DR = mybir.MatmulPerfMode.DoubleRow

H, D, E, F = 8, 256, 1536, 768
B, L = 64, 256
NCORES = 8
GB, GH = 4, 2              # batch groups x head groups
HL = H // GH               # 4 local heads
BPC = B // GB              # 16 b's per core
NBP = BPC // 2             # 8 b-pairs per core
NQ = NBP // 2              # 4 score/softmax quads (4 b's each)
EC, FC, DC = E // 128, F // 128, D // 128   # 12, 6, 2
# Per-bp padded unmasked-column counts (host sorts 64 b's by count; bp_j
# takes ranks [8j, 8j+8)).  Defaults match the fixed harness input.
LPS_DEFAULT = (152, 136, 132, 130, 128, 126, 124, 120)
SK, SB = 64.0, 16.0        # fp8 pre-scales for Key / bank (powers of two)


def _build_program(lps=LPS_DEFAULT):
    assert all(lp % 2 == 0 for lp in lps)
    lhs_ = [lp // 2 for lp in lps]     # l-chunks: two per b
    lpps = [2 * lp for lp in lps]      # (b2, l') columns per (h, dc) group
    lqs = [lhs_[2 * j] for j in range(NQ)]    # quad row counts (max of pair)
    kt_cols = 2 * FC * D               # per-h Key cols ([K8, Kr] streams)
    bkt_cols = [2 * FC * w for w in lpps]     # per-bp bankT cols
    bkt_off = np.cumsum([0] + bkt_cols).tolist()
    # bkn/sbias rows are padded to the quad max so one softmax chain can
    # cover 4 b's; pad rows carry -1e4 bias (exp -> 0) / zero bank rows
    bkn_off = np.cumsum([0] + [2 * lqs[bp // 2] for bp in range(NBP)]).tolist()
    sb_off = np.cumsum([0] + [4 * lqs[bp // 2] for bp in range(NBP)]).tolist()
    tanh_scale = 1.0 / (SK * SB)

    nc = bacc.Bacc("TRN2", target_bir_lowering=False, debug=False,
                   enable_asserts=False, num_devices=NCORES)
    qt = nc.dram_tensor("qt", [HL, 128, EC * D], F16, kind="ExternalInput").ap()
    xt = nc.dram_tensor("xt", [128, EC * BPC], F16, kind="ExternalInput").ap()
    kt = nc.dram_tensor("kt", [HL, 128, kt_cols], FP8, kind="ExternalInput").ap()
    bkt = nc.dram_tensor("bkt", [128, bkt_off[-1]], FP8, kind="ExternalInput").ap()
    bkn = nc.dram_tensor("bkn", [bkn_off[-1], 2 * F], BF16, kind="ExternalInput").ap()
    sbias = nc.dram_tensor("sbias", [1, sb_off[-1]], F32, kind="ExternalInput").ap()
    # out cols: (quad, i4=(bp2, b2), fc, h)
    out = nc.dram_tensor("out", [128, NQ * 4 * FC * HL], F16,
                         kind="ExternalOutput").ap()

    with tile.TileContext(nc) as tc:
        with tc.tile_pool(name="const", bufs=1) as cpool, \
             tc.tile_pool(name="weights", bufs=1) as wpool, \
             tc.tile_pool(name="bktp", bufs=1) as bpool, \
             tc.tile_pool(name="bknp", bufs=1) as npool, \
             tc.tile_pool(name="ksb", bufs=1) as kpool, \
             tc.tile_pool(name="small", bufs=4) as spool, \
             tc.tile_pool(name="psK", bufs=4, space="PSUM") as psK, \
             tc.tile_pool(name="psQ", bufs=1, space="PSUM") as psQ, \
             tc.tile_pool(name="psS", bufs=3, space="PSUM") as psS:

            # ---------------- SBUF tiles ----------------------------------
            xt_sb = cpool.tile([128, EC * BPC], F16)
            kt_sb = [wpool.tile([128, kt_cols], FP8, name=f"kt{h}", tag=f"kt{h}")
                     for h in range(HL)]
            qt_sb = [wpool.tile([128, EC * D], F16, name=f"qt{h}", tag=f"qt{h}")
                     for h in range(HL)]
            bkt_t = [bpool.tile([128, bkt_cols[bp]], FP8,
                                name=f"bkt{bp}", tag=f"bkt{bp}")
                     for bp in range(NBP)]
            bkn_t = [[npool.tile([lqs[bp // 2], 2 * F], BF16,
                                 name=f"bkn{bp}_{b2}", tag=f"bkn{bp}_{b2}")
                      for b2 in range(2)] for bp in range(NBP)]
            sb_sb = cpool.tile([1, sb_off[-1]], F32)
            # f16 output: final values are O(1), so f16 (0.05% rel) halves
            # the tail-critical out-DMA transfers; host upcasts
            o2a = cpool.tile([128, (NQ - 1) * 4 * FC * HL], F16)
            o2b = cpool.tile([128, 4 * FC * HL], F16)
            onesb = cpool.tile([1, BPC], F32)
            ones_mat = cpool.tile([lqs[0], 128], BF16)
            q_sb = cpool.tile([128, 128], F16)

            # ---------------- DMA: priority order -------------------------
            def dma_bkt(bp, s=None):
                o = bkt_off[bp]
                w = bkt_cols[bp]
                if s is None:
                    nc.sync.dma_start(bkt_t[bp][:], bkt[:, o:o + w])
                else:
                    h2 = w // 2
                    nc.sync.dma_start(bkt_t[bp][:, s * h2:(s + 1) * h2],
                                      bkt[:, o + s * h2:o + (s + 1) * h2])

            # kt0 K8-half and bkt0/1 B8-halves first so the warm-phase T1
            # matmuls (k0+k1 interleaved per head) start early
            hk = kt_cols // 2
            nc.sync.dma_start(kt_sb[0][:, 0:hk], kt[0, :, 0:hk])
            dma_bkt(0, 1)
            dma_bkt(1, 1)
            nc.sync.dma_start(kt_sb[0][:, hk:2 * hk], kt[0, :, hk:2 * hk])
            dma_bkt(0, 0)
            dma_bkt(1, 0)
            nc.sync.dma_start(kt_sb[1][:, 0:hk], kt[1, :, 0:hk])
            nc.sync.dma_start(kt_sb[1][:, hk:2 * hk], kt[1, :, hk:2 * hk])
            nc.sync.dma_start(xt_sb[:], xt)
            nc.vector.memset(onesb[:], 1.0)
            nc.vector.memset(ones_mat[:], 1.0)
            nc.sync.dma_start(kt_sb[2][:], kt[2])
            dma_bkt(2)
            nc.sync.dma_start(kt_sb[3][:], kt[3])
            dma_bkt(3)
            nc.sync.dma_start(qt_sb[0][:], qt[0])
            dma_bkt(4)
            nc.sync.dma_start(qt_sb[1][:], qt[1])
            dma_bkt(5)
            nc.sync.dma_start(qt_sb[2][:], qt[2])
            nc.sync.dma_start(qt_sb[3][:], qt[3])
            nc.sync.dma_start(sb_sb[:], sbias)
            dma_bkt(6)
            dma_bkt(7)
            for bp in range(NBP):
                for b2 in range(2):
                    r = bkn_off[bp] + b2 * lqs[bp // 2]
                    nc.sync.dma_start(bkn_t[bp][b2][:],
                                      bkn[r:r + lqs[bp // 2]])

            # ---------------- k = tanh(Key @ bankT), all bps --------------
            k_sb = {}

            def k_phase(bps, warm=False, post_h=None, tail_quad=None):
                def t1_mms(bp, h, ps):
                    lpp = lpps[bp]
                    vb = bkt_t[bp][:].rearrange("p (s ft c) -> p s ft c",
                                                s=2, ft=FC)
                    vk = kt_sb[h][:].rearrange("p (s ft d) -> p s ft d",
                                               s=2, ft=FC)
                    for dc in range(DC):
                        g = ps[dc][:, 0:lpp]
                        for p in range(FC // 2):
                            nc.tensor.matmul(
                                g,
                                vk[:, 0, 2 * p:2 * p + 2,
                                   dc * 128:(dc + 1) * 128],
                                vb[:, 1, 2 * p:2 * p + 2],
                                start=(p == 0), stop=False, perf_mode=DR)

                def cross_evict(bp, h, ps):
                    lpp = lpps[bp]
                    vb = bkt_t[bp][:].rearrange("p (s ft c) -> p s ft c",
                                                s=2, ft=FC)
                    vk = kt_sb[h][:].rearrange("p (s ft d) -> p s ft d",
                                               s=2, ft=FC)
                    for dc in range(DC):
                        g = ps[dc][:, 0:lpp]
                        # cross terms: K8.Br + Kr.B8 per f-tile
                        for ft in range(FC):
                            nc.tensor.matmul(
                                g, vk[:, :, ft, dc * 128:(dc + 1) * 128],
                                vb[:, :, ft],
                                start=False, stop=(ft == FC - 1),
                                perf_mode=DR)
                        # per-dc eviction into a per-dc tile: dc0 drains
                        # while dc1 fills, and (deps being tile-granular)
                        # score matmuls of dc0 can fire before the dc1
                        # eviction lands
                        kt_out = kpool.tile([128, lpp], F16,
                                            name=f"k{bp}_{h}_{dc}",
                                            tag=f"k{bp}_{h}_{dc}")
                        nc.scalar.activation(
                            kt_out[:, 0:lpp],
                            ps[dc][:, 0:lpp],
                            AF.Tanh, scale=tanh_scale)
                        k_sb[(bp, h, dc)] = kt_out

                start_h = 0
                if warm:
                    # four T1 groups lead (they need only the K8/B8
                    # slices; the residual streams for the crosses land
                    # while they run).  The third group borrows a psS
                    # buffer pair (idle until the score phase).
                    b0, b1 = bps[0], bps[1]
                    tA = [psK.tile([128, 512], F32, name="psk", tag="psk")
                          for _ in range(DC)]
                    t1_mms(b0, 0, tA)
                    tB = [psK.tile([128, 512], F32, name="psk", tag="psk")
                          for _ in range(DC)]
                    t1_mms(b1, 0, tB)
                    tC = [psS.tile([128, 512], F32, name="mix", tag="mix")
                          for _ in range(DC)]
                    t1_mms(b0, 1, tC)
                    cross_evict(b0, 0, tA)
                    tD = [psK.tile([128, 512], F32, name="psk", tag="psk")
                          for _ in range(DC)]
                    t1_mms(b1, 1, tD)
                    cross_evict(b1, 0, tB)
                    cross_evict(b0, 1, tC)
                    cross_evict(b1, 1, tD)
                    start_h = 2
                for h in range(start_h, HL):
                    for bp in bps:
                        ps = [psK.tile([128, 512], F32,
                                       name="psk", tag="psk")
                              for _ in range(DC)]
                        t1_mms(bp, h, ps)
                        cross_evict(bp, h, ps)
                    if tail_quad is not None:
                        # the quad's score matmuls ride one head behind
                        # this phase's own evictions; a bp whose k phase
                        # already finished (solo-phase case) fires at the
                        # current head, leaving less tail work
                        qd, qps = tail_quad
                        for bp2 in range(2):
                            own = (2 * qd + bp2) in bps
                            if own and h >= 1:
                                score_qh(qd, qps, bp2, h - 1)
                            elif not own:
                                score_qh(qd, qps, bp2, h)
                    if post_h and h in post_h:
                        post_h[h]()

            # k0+k1 interleaved per head (2.3us of PE work per kt[h]
            # arrival so the lead-in is never DMA-starved)
            k_phase([0, 1], warm=True)
            k_phase([2])
            k_phase([3])

            # ---------------- q = tanh(Query @ x), transposed -------------
            # qt (3.15 MB) has streamed in behind the k inputs by now
            psq = psQ.tile([128, 512], F32)
            for h in range(HL):
                vq = qt_sb[h][:].rearrange("p (ec d) -> p ec d", ec=EC)
                for dc in range(DC):
                    g = psq[:, (h * DC + dc) * BPC:(h * DC + dc + 1) * BPC]
                    for ec in range(EC):
                        nc.tensor.matmul(
                            g, vq[:, ec, dc * 128:(dc + 1) * 128],
                            xt_sb[:, ec * BPC:(ec + 1) * BPC],
                            start=(ec == 0), stop=(ec == EC - 1))
            nc.scalar.activation(q_sb[:], psq[:, 0:128], AF.Tanh)

            # ---------------- score / softmax / emb per QUAD --------------
            # One chain covers 4 b's (bps 2j, 2j+1), rows padded to the
            # quad max (pad rows get -1e4 bias -> exp 0, zero bank rows).
            # score cols: (i4=(bp2, b2), lc, h) -> 8 groups of HL
            def score_qh(qd, ps, bp2, h):
                """One head's score matmuls for one bp of a quad (+bias
                matmuls at h==0)."""
                lq = lqs[qd]
                bp = 2 * qd + bp2
                lh, lp, lpp = lhs_[bp], lps[bp], lpps[bp]
                for b2 in range(2):
                    i4 = bp2 * 2 + b2
                    for lc in range(2):
                        col = (i4 * 2 + lc) * HL
                        if h == 0:
                            boff = sb_off[bp] + (b2 * 2 + lc) * lq
                            nc.tensor.matmul(ps[0:lq, col:col + HL],
                                             sb_sb[:, boff:boff + lq],
                                             onesb[:, 0:HL],
                                             start=True, stop=False)
                        for dc in range(DC):
                            nc.tensor.matmul(
                                ps[0:lh, col + h:col + h + 1],
                                k_sb[(bp, h, dc)][:, b2 * lp + lc * lh:
                                                  b2 * lp + lc * lh + lh],
                                q_sb[:, (h * DC + dc) * BPC + bp * 2 + b2:
                                     (h * DC + dc) * BPC + bp * 2 + b2 + 1],
                                start=False,
                                stop=(h == HL - 1 and dc == DC - 1))

            def score_exp(qd, ps):
                lq = lqs[qd]
                exp_t = spool.tile([lqs[0], 8 * HL], BF16,
                                   name="exp", tag="exp")
                nc.scalar.activation(exp_t[0:lq, :], ps[0:lq, 0:8 * HL],
                                     AF.Exp)
                return ps, exp_t

            def score_part(qd, ps_tile=None):
                ps = (ps_tile if ps_tile is not None
                      else psS.tile([128, 512], F32, name="mix", tag="mix"))
                for bp2 in range(2):
                    for h in range(HL):
                        score_qh(qd, ps, bp2, h)
                return score_exp(qd, ps)

            def rest_part(qd, ps, exp_t):
                lq = lqs[qd]
                # z[i4, h] (cols 64:80): the ones-MATRIX lhsT emits column
                # sums broadcast across all 128 partitions
                ev = exp_t[0:lq, :].rearrange("p (i4 lc h) -> p i4 lc h",
                                              i4=4, lc=2)
                for lc in range(2):
                    nc.tensor.matmul(ps[:, 64:64 + 4 * HL],
                                     ones_mat[0:lq, :], ev[:, :, lc],
                                     start=(lc == 0), stop=(lc == 1))
                rz = spool.tile([128, 4 * HL], F32, name="rz", tag="rz")
                nc.vector.reciprocal(rz[:], ps[:, 64:64 + 4 * HL])
                # emb[f, (i4, fc, h)] (cols 192:288)
                for bp2 in range(2):
                    bp = 2 * qd + bp2
                    for b2 in range(2):
                        i4 = bp2 * 2 + b2
                        for fc in range(FC):
                            col = 192 + (i4 * FC + fc) * HL
                            for lc in range(2):
                                nc.tensor.matmul(
                                    ps[:, col:col + HL],
                                    bkn_t[bp][b2][:, lc * F + fc * 128:
                                                  lc * F + fc * 128 + 128],
                                    exp_t[0:lq, (i4 * 2 + lc) * HL:
                                          (i4 * 2 + lc + 1) * HL],
                                    start=(lc == 0), stop=(lc == 1))
                o1 = spool.tile([128, 4 * FC * HL], F32, name="o1", tag="o1")
                w = 4 * FC * HL
                o2 = (o2b[:, 0:w] if qd == NQ - 1
                      else o2a[:, qd * w:(qd + 1) * w])
                # LeakyReLU commutes with the positive 1/z: Prelu the raw
                # emb on Act in parallel with the z/recip chain; one DVE
                # multiply finishes
                nc.scalar.activation(o1[:], ps[:, 192:192 + w], AF.Prelu,
                                     alpha=0.4)
                vb = rz[:].rearrange(
                    "p (i4 one h) -> p i4 one h", i4=4,
                    one=1).broadcast_to([128, 4, FC, HL])
                nc.vector.tensor_mul(
                    o2.rearrange("p (i4 fc h) -> p i4 fc h", i4=4, fc=FC),
                    o1[:].rearrange("p (i4 fc h) -> p i4 fc h", i4=4, fc=FC),
                    vb)
                if qd == NQ - 2:
                    # quads 0-2 stream out while quad3 still computes
                    nc.sync.dma_start(out[:, 0:(NQ - 1) * w], o2a[:])
                elif qd == NQ - 1:
                    # only quad3's small transfer sits on the tail
                    nc.sync.dma_start(out[:, (NQ - 1) * w:NQ * w], o2b[:])

            # quads 0/1 score+softmax overlap k4/k5; quad2's inputs finish
            # at k5 so its chain rides inside the joint k6+k7 phase; quad3
            # scores after it (pending/rest pipeline shape)
            k_phase([4])
            # quad0/1 scores issue right after their last input phase so
            # their softmax chains drain a phase earlier; quad2's scores
            # ride inside k5 (bp4's k is done, bp5 one head behind)
            cur0 = (0, *score_part(0))
            ps2t = psS.tile([128, 512], F32, name="mix", tag="mix")
            k_phase([5], tail_quad=(2, ps2t))
            cur1 = (1, *score_part(1))
            rest_part(*cur0)
            score_qh(2, ps2t, 1, HL - 1)
            cur2 = (2, *score_exp(2, ps2t))
            rest_part(*cur1)
            k_phase([6, 7], post_h={1: (lambda: rest_part(*cur2))},
                    tail_quad=(NQ - 1, psq))
            # tail: last head's scores, exp, softmax/emb/out for quad3
            for bp2 in range(2):
                score_qh(NQ - 1, psq, bp2, HL - 1)
            rest_part(NQ - 1, *score_exp(NQ - 1, psq))

    nc.finalize()
    return nc


def _slot_plan(mask):
    """Sort b's by unmasked count (desc); bp_j takes ranks [8j, 8j+8).
    Returns (perm, lps): perm[slot] = original b, slot = gb*BPC + j*2 + b2."""
    counts = mask.sum(axis=1)
    order = np.argsort(-counts, kind="stable")
    perm = np.empty(B, dtype=np.int64)
    for j in range(NBP):
        grp = order[8 * j:8 * (j + 1)]
        for gb in range(GB):
            perm[gb * BPC + j * 2] = grp[2 * gb]
            perm[gb * BPC + j * 2 + 1] = grp[2 * gb + 1]
    lps = tuple(max(int(2 * ((counts[order[8 * j]] + 1) // 2)), 8)
                for j in range(NBP))
    return perm, lps


def _host_prep(x, bank, mask, Query, Key, perm, lps):
    x = np.asarray(x, dtype=np.float32)
    bank = np.asarray(bank, dtype=np.float32)
    mask = np.asarray(mask)
    Query = np.asarray(Query, dtype=np.float32)
    Key = np.asarray(Key, dtype=np.float32)
    e4 = ml_dtypes.float8_e4m3
    lhs_ = [lp // 2 for lp in lps]
    lqs = [lhs_[2 * j] for j in range(len(lps) // 2)]

    # q path: f16, host-transposed; per head-group slice
    xs = x[perm]
    qt_full = np.ascontiguousarray(Query.transpose(0, 2, 1)).reshape(
        H, EC, 128, D).transpose(0, 2, 1, 3).reshape(H, 128, EC * D)
    qt_full = qt_full.astype(np.float16)

    Ks = Key * SK
    K8 = Ks.astype(e4)
    Kr = (Ks - K8.astype(np.float32)).astype(e4)

    def swz_key(Kt):  # [H, D, F] -> [H, 128(f), FC, D]
        t = np.ascontiguousarray(Kt.transpose(0, 2, 1))
        return t.reshape(H, FC, 128, D).transpose(0, 2, 1, 3)

    kt_full = np.stack([swz_key(K8.astype(np.float32)),
                        swz_key(Kr.astype(np.float32))], axis=2)
    kt_full = kt_full.reshape(H, 128, 2 * FC * D).astype(e4)

    # per-(batch-group, bp) compacted bank streams
    bkt_cols = sum(2 * FC * 2 * lp for lp in lps)
    gb_data = []
    for gb in range(GB):
        bkt_c = np.zeros((128, bkt_cols), dtype=e4)
        bkn_rows = []
        sb_c = []
        col = 0
        for j in range(NBP):
            lp, lh, lq = lps[j], lhs_[j], lqs[j // 2]
            bc = np.zeros((2, lp, F), dtype=np.float32)
            # bias rows padded to the quad max (pad rows -> -1e4)
            bias = np.full((2, 2, lq), -10000.0, dtype=np.float32)
            for b2 in range(2):
                bsrc = perm[gb * BPC + j * 2 + b2]
                idx = np.nonzero(mask[bsrc])[0]
                bc[b2, :len(idx)] = bank[bsrc, idx]
                # column c of this b maps to row c%lh, chunk lc=c//lh;
                # valid rows per lc chunk:
                n = len(idx)
                for lc in range(2):
                    good = max(0, min(lh, n - lc * lh))
                    bias[b2, lc, :good] = 0.0
            # bankT swizzle: [2, lp, F] -> [128(f), s, FC, 2, lp]
            t = np.ascontiguousarray(bc.transpose(0, 2, 1))     # [2, F, lp]
            t = t.reshape(2, FC, 128, lp).transpose(2, 1, 0, 3)  # [128,FC,2,lp]
            ts = t * SB
            t8 = ts.astype(e4)
            tr = (ts - t8.astype(np.float32)).astype(e4)
            blk = np.stack([tr, t8.astype(e4)], axis=1).reshape(
                128, 2 * FC * 2 * lp)
            w = 2 * FC * 2 * lp
            bkt_c[:, col:col + w] = blk
            col += w
            # bkn rows [2*lq, 2F] per bp (b2-major): rows beyond lh zero
            br = np.zeros((2, lq, 2 * F), dtype=np.float32)
            bcq = bc.reshape(2, 2, lh, F)   # [b2, lc, lh, F]
            br[:, :lh, 0:F] = bcq[:, 0]
            br[:, :lh, F:2 * F] = bcq[:, 1]
            bkn_rows.append(br.reshape(2 * lq, 2 * F))
            sb_c.append(bias.reshape(4 * lq))
        xt_gb = np.ascontiguousarray(
            xs[gb * BPC:(gb + 1) * BPC].T.reshape(EC, 128, BPC)
            .transpose(1, 0, 2).reshape(128, EC * BPC)).astype(np.float16)
        gb_data.append({
            "xt": xt_gb,
            "bkt": bkt_c,
            "bkn": np.ascontiguousarray(np.concatenate(bkn_rows, axis=0))
            .astype(ml_dtypes.bfloat16),
            "sbias": np.concatenate(sb_c)[None, :].astype(np.float32),
        })

    in_maps = []
    for c in range(NCORES):
        gb, gh = c // GH, c % GH
        m = dict(gb_data[gb])
        m["qt"] = qt_full[gh * HL:(gh + 1) * HL]
        m["kt"] = kt_full[gh * HL:(gh + 1) * HL]
        in_maps.append(m)
    return in_maps


_NC_CACHE = {}


def kernel(x, bank, mask, Query, Key):
    mask = np.asarray(mask)
    perm, lps = _slot_plan(mask)
    if lps not in _NC_CACHE:
        _NC_CACHE[lps] = _build_program(lps)
    nc = _NC_CACHE[lps]
    in_maps = _host_prep(x, bank, mask, Query, Key, perm, lps)

    trace = os.environ.get("KERNEL_TRACE", "0") == "1"
    res = bass_utils.run_bass_kernel_spmd(nc, in_maps,
                                          core_ids=list(range(NCORES)),
                                          trace=trace)
    full = np.empty((B, H, F), dtype=np.float32)
    for c, r in enumerate(res.results):
        gb, gh = c // GH, c % GH
        a = r["out"].astype(np.float32).reshape(128, NQ, 2, 2, FC, HL)
        # [p, quad, bp2, b2, fc, h] -> slot (j=quad*2+bp2, b2) -> [BPC,HL,F]
        a = a.transpose(1, 2, 3, 5, 4, 0).reshape(BPC, HL, F)
        full[perm[gb * BPC:(gb + 1) * BPC], gh * HL:(gh + 1) * HL] = a
    return np.ascontiguousarray(full)
